# revision 2
# baseline (speedup 1.0000x reference)
"""Trainium2 Bass kernel for nn_AdaMLP (MoE routing, 64 experts, 2-layer MLP).

Strategy: expert-parallel over 8 NeuronCores. Core i owns experts
[8i, 8i+8). The host groups slots by expert index (the MoE dispatch),
pads each group to a common capacity C, and ships each core:
  - its 8 experts' weights, pre-rearranged into the SBUF layout the
    TensorEngine wants (one contiguous [128, 4096] block per expert),
  - the transposed slot groups xT [256, 8*C],
  - biases packed per-partition.
Each core computes, per expert:  H^T = W1^T-blocks @ xT (relu+b1),
Out^T = W2-blocks @ H^T (+b2), with the weights as the stationary
matmul operand so each weight element crosses the PE exactly once.
The host scatters per-slot outputs back to the full (B,K,D) output.

Total HBM traffic is one pass over the 128 MB weight tables, perfectly
balanced at 16 MB/core — the memory roofline for this problem.
"""

import numpy as np

P = 128                    # SBUF partitions
DIM = 256                  # slot dim
R = 1024                   # hidden dim
E = 64                     # num experts
NCORES = 8
EPC = E // NCORES          # experts per core
DC = DIM // P              # layer-1 contraction chunks (2)
RC = R // P                # r chunks (8)
OC = DIM // P              # output dim chunks (2)
WCOLS = DC * R + RC * DIM  # weight columns per expert (4096)
BPE = RC + OC              # bias columns per expert (10)

USE_BF16 = False

_GRAPH_CACHE: dict = {}


def _build_graph(C: int, use_bf16: bool):
    import concourse.bacc as bacc
    import concourse.tile as tile
    from concourse import mybir

    f32 = mybir.dt.float32
    cdt = mybir.dt.bfloat16 if use_bf16 else f32

    # SBUF budget shrinks as the pad capacity C grows (pathological skew).
    if C <= 128:
        wg_bufs, h_bufs = 4, 2
    elif C <= 512:
        wg_bufs, h_bufs = 2, 2
    else:
        wg_bufs, h_bufs = 1, 1

    nc = bacc.Bacc(None, target_bir_lowering=False)
    xt_ext = nc.declare_dram_parameter("xt", [P, DC * EPC * C], cdt, isOutput=False)
    wg_ext = nc.declare_dram_parameter("wg", [EPC, P, WCOLS], cdt, isOutput=False)
    bias_ext = nc.declare_dram_parameter("bias", [P, EPC * BPE], f32, isOutput=False)
    out_ext = nc.declare_dram_parameter("out", [P, EPC * OC * C], f32, isOutput=True)

    mm = mybir.AluOpType.max
    aa = mybir.AluOpType.add

    with tile.TileContext(nc) as tc:
        with (
            tc.tile_pool(name="wpool", bufs=wg_bufs) as wpool,
            tc.tile_pool(name="xpool", bufs=1) as xpool,
            tc.tile_pool(name="hpool", bufs=h_bufs) as hpool,
            tc.tile_pool(name="opool", bufs=2) as opool,
            tc.tile_pool(name="pspool", bufs=4, space="PSUM") as pspool,
        ):
            xt = xpool.tile([P, DC * EPC * C], cdt)
            nc.gpsimd.dma_start(xt[:], xt_ext[:])
            bias = xpool.tile([P, EPC * BPE], f32)
            nc.gpsimd.dma_start(bias[:], bias_ext[:])

            # weight loads alternate between the Sync and Scalar HWDGE
            # queues so one ring's descriptors flow while the other's
            # DMA retires; expert 0 is split w1|w2 so layer 1 can start
            # after half the bytes land.
            wgs = []
            for e in range(EPC):
                wg = wpool.tile([P, WCOLS], cdt)
                eng = nc.sync if e % 2 == 0 else nc.scalar
                if e == 0:
                    nc.sync.dma_start(wg[:, : DC * R], wg_ext[0, :, : DC * R])
                    nc.scalar.dma_start(wg[:, DC * R :], wg_ext[0, :, DC * R :])
                else:
                    eng.dma_start(wg[:], wg_ext[e])
                wgs.append(wg)

            for e in range(EPC):
                wg = wgs[e]
                h = hpool.tile([P, RC * C], cdt)
                out_sb = opool.tile([P, OC * C], f32)
                for c0 in range(0, C, 512):
                    cw = min(512, C - c0)
                    # layer 1: H^T[r,:] = sum_d W1[d, r-block] . xT[d, :]
                    for rc_i in range(RC):
                        ps = pspool.tile([P, cw], f32)
                        for dc_i in range(DC):
                            nc.tensor.matmul(
                                ps[:],
                                wg[:, dc_i * R + rc_i * P : dc_i * R + rc_i * P + P],
                                xt[:, (dc_i * EPC + e) * C + c0 : (dc_i * EPC + e) * C + c0 + cw],
                                start=(dc_i == 0),
                                stop=(dc_i == DC - 1),
                            )
                        # relu(x + b1) on the (otherwise idle) Vector engine
                        nc.vector.tensor_scalar(
                            h[:, rc_i * C + c0 : rc_i * C + c0 + cw],
                            ps[:],
                            bias[:, e * BPE + rc_i : e * BPE + rc_i + 1],
                            0.0,
                            aa,
                            mm,
                        )
                    # layer 2: Out^T[dim,:] = sum_r W2[r, dim-block] . H^T[r, :]
                    for oc_i in range(OC):
                        ps2 = pspool.tile([P, cw], f32)
                        for rc_i in range(RC):
                            nc.tensor.matmul(
                                ps2[:],
                                wg[:, DC * R + rc_i * DIM + oc_i * P : DC * R + rc_i * DIM + oc_i * P + P],
                                h[:, rc_i * C + c0 : rc_i * C + c0 + cw],
                                start=(rc_i == 0),
                                stop=(rc_i == RC - 1),
                            )
                        nc.scalar.activation(
                            out_sb[:, oc_i * C + c0 : oc_i * C + c0 + cw],
                            ps2[:],
                            mybir.ActivationFunctionType.Identity,
                            bias=bias[:, e * BPE + RC + oc_i : e * BPE + RC + oc_i + 1],
                        )
                nc.gpsimd.dma_start(out_ext[:, e * OC * C : (e + 1) * OC * C], out_sb[:])
    nc.compile()
    return nc


def _get_graph(C: int, use_bf16: bool):
    key = (C, use_bf16)
    if key not in _GRAPH_CACHE:
        _GRAPH_CACHE[key] = _build_graph(C, use_bf16)
    return _GRAPH_CACHE[key]


def _run(inputs: dict, trace: bool = False, trace_cores=None, use_bf16=None, **spmd_kwargs):
    from concourse.bass_utils import run_bass_kernel_spmd

    if use_bf16 is None:
        use_bf16 = USE_BF16
    if use_bf16:
        import ml_dtypes
        cdt_np = ml_dtypes.bfloat16
    else:
        cdt_np = np.float32

    slots = np.asarray(inputs["slots"], np.float32)
    w1 = np.asarray(inputs["w1"], np.float32)
    b1 = np.asarray(inputs["b1"], np.float32)
    w2 = np.asarray(inputs["w2"], np.float32)
    b2 = np.asarray(inputs["b2"], np.float32)
    indices = np.asarray(inputs["indices"]).astype(np.int64)

    B, K, D = slots.shape
    assert D == DIM and w1.shape == (E, DIM, R) and w2.shape == (E, R, DIM)
    X = slots.reshape(B * K, DIM)
    idx = indices.reshape(B * K)

    counts = np.bincount(idx, minlength=E)
    C = max(int(counts.max()), 16)
    C = ((C + 15) // 16) * 16  # stable capacities -> stable NEFF cache keys

    in_maps = []
    pos_lists = []
    for core in range(NCORES):
        xt = np.zeros((P, DC * EPC * C), cdt_np)
        wg = np.empty((EPC, P, WCOLS), cdt_np)
        bias = np.zeros((P, EPC * BPE), np.float32)
        core_pos = []
        for e in range(EPC):
            g = core * EPC + e
            pos = np.nonzero(idx == g)[0]
            core_pos.append(pos)
            n = len(pos)
            if n:
                xeT = X[pos].T.astype(cdt_np)  # [DIM, n]
                for dc_i in range(DC):
                    xt[:, (dc_i * EPC + e) * C : (dc_i * EPC + e) * C + n] = (
                        xeT[dc_i * P : (dc_i + 1) * P]
                    )
            wg[e, :, : DC * R] = (
                w1[g].reshape(DC, P, R).transpose(1, 0, 2).reshape(P, DC * R)
            )
            wg[e, :, DC * R :] = (
                w2[g].reshape(RC, P, DIM).transpose(1, 0, 2).reshape(P, RC * DIM)
            )
            bias[:, e * BPE : e * BPE + RC] = b1[g].reshape(RC, P).T
            bias[:, e * BPE + RC : (e + 1) * BPE] = b2[g].reshape(OC, P).T
        in_maps.append({"xt": xt, "wg": wg, "bias": bias})
        pos_lists.append(core_pos)

    nc = _get_graph(C, use_bf16)
    res = run_bass_kernel_spmd(
        nc, in_maps, core_ids=list(range(NCORES)), trace=trace,
        trace_cores=trace_cores, **spmd_kwargs,
    )

    out_flat = np.zeros((B * K, DIM), np.float32)
    for core in range(NCORES):
        o = res.results[core]["out"]  # [P, EPC*OC*C]
        for e in range(EPC):
            pos = pos_lists[core][e]
            n = len(pos)
            if n == 0:
                continue
            blk = np.empty((n, DIM), np.float32)
            for oc_i in range(OC):
                cols = o[:, (e * OC + oc_i) * C : (e * OC + oc_i) * C + n]
                blk[:, oc_i * P : (oc_i + 1) * P] = cols.T
            out_flat[pos] = blk
    return out_flat.reshape(B, K, DIM), res


def kernel(**inputs) -> np.ndarray:
    out, _ = _run(inputs)
    return out


# revision 4
# speedup vs baseline: 1.0807x; 1.0807x over previous
"""Trainium2 Bass kernel for nn_AdaMLP (MoE routing, 64 experts, 2-layer MLP).

Strategy: expert-parallel over 8 NeuronCores. Core i owns experts
[8i, 8i+8). The host groups slots by expert index (the MoE dispatch),
pads each group to a common capacity C, and ships each core:
  - its 8 experts' weights, pre-rearranged into the SBUF layout the
    TensorEngine wants (one contiguous [128, 4096] block per expert),
  - the transposed slot groups xT [256, 8*C],
  - biases packed per-partition.
Each core computes, per expert:  H^T = W1^T-blocks @ xT (relu+b1),
Out^T = W2-blocks @ H^T (+b2), with the weights as the stationary
matmul operand so each weight element crosses the PE exactly once.
The host scatters per-slot outputs back to the full (B,K,D) output.

Total HBM traffic is one pass over the 128 MB weight tables, perfectly
balanced at 16 MB/core — the memory roofline for this problem.
"""

import numpy as np

P = 128                    # SBUF partitions
DIM = 256                  # slot dim
R = 1024                   # hidden dim
E = 64                     # num experts
NCORES = 8
EPC = E // NCORES          # experts per core
DC = DIM // P              # layer-1 contraction chunks (2)
RC = R // P                # r chunks (8)
OC = DIM // P              # output dim chunks (2)
WCOLS = DC * R + RC * DIM  # weight columns per expert (4096)
BPE = RC + OC              # bias columns per expert (10)

USE_BF16 = False

_GRAPH_CACHE: dict = {}


def _build_graph(C: int, use_bf16: bool):
    import concourse.bacc as bacc
    import concourse.tile as tile
    from concourse import mybir

    f32 = mybir.dt.float32
    cdt = mybir.dt.bfloat16 if use_bf16 else f32

    # SBUF budget shrinks as the pad capacity C grows (pathological skew).
    if C <= 128:
        wg_bufs, h_bufs = 5, 2
    elif C <= 512:
        wg_bufs, h_bufs = 2, 2
    else:
        wg_bufs, h_bufs = 1, 1

    nc = bacc.Bacc(None, target_bir_lowering=False)
    xt_ext = nc.declare_dram_parameter("xt", [P, DC * EPC * C], cdt, isOutput=False)
    wg_ext = nc.declare_dram_parameter("wg", [EPC, P, WCOLS], cdt, isOutput=False)
    bias_ext = nc.declare_dram_parameter("bias", [P, EPC * BPE], f32, isOutput=False)
    out_ext = nc.declare_dram_parameter("out", [P, EPC * OC * C], f32, isOutput=True)

    mm = mybir.AluOpType.max
    aa = mybir.AluOpType.add

    with tile.TileContext(nc) as tc:
        with (
            tc.tile_pool(name="wpool", bufs=wg_bufs) as wpool,
            tc.tile_pool(name="xpool", bufs=1) as xpool,
            tc.tile_pool(name="hpool", bufs=h_bufs) as hpool,
            tc.tile_pool(name="opool", bufs=2) as opool,
            tc.tile_pool(name="pspool", bufs=4, space="PSUM") as pspool,
        ):
            # One HWDGE queue (Sync) for all loads: queue order is the
            # critical path — xt first (first matmul's rhs), then expert
            # 0's w1 half so layer 1 starts after half its bytes land.
            xt = xpool.tile([P, DC * EPC * C], cdt)
            nc.sync.dma_start(xt[:], xt_ext[:])
            bias = xpool.tile([P, EPC * BPE], f32)
            wgs = []
            for e in range(EPC):
                wg = wpool.tile([P, WCOLS], cdt)
                if e == 0:
                    nc.sync.dma_start(wg[:, : DC * R], wg_ext[0, :, : DC * R])
                    nc.sync.dma_start(wg[:, DC * R :], wg_ext[0, :, DC * R :])
                    nc.sync.dma_start(bias[:], bias_ext[:])
                else:
                    nc.sync.dma_start(wg[:], wg_ext[e])
                wgs.append(wg)

            for e in range(EPC):
                wg = wgs[e]
                h = hpool.tile([P, RC * C], cdt)
                out_sb = opool.tile([P, OC * C], f32)
                for c0 in range(0, C, 512):
                    cw = min(512, C - c0)
                    # layer 1: H^T[r,:] = sum_d W1[d, r-block] . xT[d, :]
                    for rc_i in range(RC):
                        ps = pspool.tile([P, cw], f32)
                        for dc_i in range(DC):
                            nc.tensor.matmul(
                                ps[:],
                                wg[:, dc_i * R + rc_i * P : dc_i * R + rc_i * P + P],
                                xt[:, (dc_i * EPC + e) * C + c0 : (dc_i * EPC + e) * C + c0 + cw],
                                start=(dc_i == 0),
                                stop=(dc_i == DC - 1),
                            )
                        # relu(x + b1) on the (otherwise idle) Vector engine
                        nc.vector.tensor_scalar(
                            h[:, rc_i * C + c0 : rc_i * C + c0 + cw],
                            ps[:],
                            bias[:, e * BPE + rc_i : e * BPE + rc_i + 1],
                            0.0,
                            aa,
                            mm,
                        )
                    # layer 2: Out^T[dim,:] = sum_r W2[r, dim-block] . H^T[r, :]
                    for oc_i in range(OC):
                        ps2 = pspool.tile([P, cw], f32)
                        for rc_i in range(RC):
                            nc.tensor.matmul(
                                ps2[:],
                                wg[:, DC * R + rc_i * DIM + oc_i * P : DC * R + rc_i * DIM + oc_i * P + P],
                                h[:, rc_i * C + c0 : rc_i * C + c0 + cw],
                                start=(rc_i == 0),
                                stop=(rc_i == RC - 1),
                            )
                        nc.scalar.activation(
                            out_sb[:, oc_i * C + c0 : oc_i * C + c0 + cw],
                            ps2[:],
                            mybir.ActivationFunctionType.Identity,
                            bias=bias[:, e * BPE + RC + oc_i : e * BPE + RC + oc_i + 1],
                        )
                nc.gpsimd.dma_start(out_ext[:, e * OC * C : (e + 1) * OC * C], out_sb[:])
    nc.compile()
    return nc


def _get_graph(C: int, use_bf16: bool):
    key = (C, use_bf16)
    if key not in _GRAPH_CACHE:
        _GRAPH_CACHE[key] = _build_graph(C, use_bf16)
    return _GRAPH_CACHE[key]


def _run(inputs: dict, trace: bool = False, trace_cores=None, use_bf16=None, **spmd_kwargs):
    from concourse.bass_utils import run_bass_kernel_spmd

    if use_bf16 is None:
        use_bf16 = USE_BF16
    if use_bf16:
        import ml_dtypes
        cdt_np = ml_dtypes.bfloat16
    else:
        cdt_np = np.float32

    slots = np.asarray(inputs["slots"], np.float32)
    w1 = np.asarray(inputs["w1"], np.float32)
    b1 = np.asarray(inputs["b1"], np.float32)
    w2 = np.asarray(inputs["w2"], np.float32)
    b2 = np.asarray(inputs["b2"], np.float32)
    indices = np.asarray(inputs["indices"]).astype(np.int64)

    B, K, D = slots.shape
    assert D == DIM and w1.shape == (E, DIM, R) and w2.shape == (E, R, DIM)
    X = slots.reshape(B * K, DIM)
    idx = indices.reshape(B * K)

    counts = np.bincount(idx, minlength=E)
    C = max(int(counts.max()), 16)
    C = ((C + 15) // 16) * 16  # stable capacities -> stable NEFF cache keys

    in_maps = []
    pos_lists = []
    for core in range(NCORES):
        xt = np.zeros((P, DC * EPC * C), cdt_np)
        wg = np.empty((EPC, P, WCOLS), cdt_np)
        bias = np.zeros((P, EPC * BPE), np.float32)
        core_pos = []
        for e in range(EPC):
            g = core * EPC + e
            pos = np.nonzero(idx == g)[0]
            core_pos.append(pos)
            n = len(pos)
            if n:
                xeT = X[pos].T.astype(cdt_np)  # [DIM, n]
                for dc_i in range(DC):
                    xt[:, (dc_i * EPC + e) * C : (dc_i * EPC + e) * C + n] = (
                        xeT[dc_i * P : (dc_i + 1) * P]
                    )
            wg[e, :, : DC * R] = (
                w1[g].reshape(DC, P, R).transpose(1, 0, 2).reshape(P, DC * R)
            )
            wg[e, :, DC * R :] = (
                w2[g].reshape(RC, P, DIM).transpose(1, 0, 2).reshape(P, RC * DIM)
            )
            bias[:, e * BPE : e * BPE + RC] = b1[g].reshape(RC, P).T
            bias[:, e * BPE + RC : (e + 1) * BPE] = b2[g].reshape(OC, P).T
        in_maps.append({"xt": xt, "wg": wg, "bias": bias})
        pos_lists.append(core_pos)

    nc = _get_graph(C, use_bf16)
    res = run_bass_kernel_spmd(
        nc, in_maps, core_ids=list(range(NCORES)), trace=trace,
        trace_cores=trace_cores, **spmd_kwargs,
    )

    out_flat = np.zeros((B * K, DIM), np.float32)
    for core in range(NCORES):
        o = res.results[core]["out"]  # [P, EPC*OC*C]
        for e in range(EPC):
            pos = pos_lists[core][e]
            n = len(pos)
            if n == 0:
                continue
            blk = np.empty((n, DIM), np.float32)
            for oc_i in range(OC):
                cols = o[:, (e * OC + oc_i) * C : (e * OC + oc_i) * C + n]
                blk[:, oc_i * P : (oc_i + 1) * P] = cols.T
            out_flat[pos] = blk
    return out_flat.reshape(B, K, DIM), res


def kernel(**inputs) -> np.ndarray:
    out, _ = _run(inputs)
    return out


# revision 5
# speedup vs baseline: 1.1506x; 1.0647x over previous
"""Trainium2 Bass kernel for nn_AdaMLP (MoE routing, 64 experts, 2-layer MLP).

Strategy: expert-parallel over 8 NeuronCores. Core i owns experts
[8i, 8i+8). The host groups slots by expert index (the MoE dispatch),
pads each group to a common capacity C, and ships each core:
  - its 8 experts' weights, pre-rearranged into the SBUF layout the
    TensorEngine wants (one contiguous [128, 4096] block per expert),
  - the transposed slot groups xT [256, 8*C],
  - biases packed per-partition.
Each core computes, per expert:  H^T = W1^T-blocks @ xT (relu+b1),
Out^T = W2-blocks @ H^T (+b2), with the weights as the stationary
matmul operand so each weight element crosses the PE exactly once.
The host scatters per-slot outputs back to the full (B,K,D) output.

Total HBM traffic is one pass over the 128 MB weight tables, perfectly
balanced at 16 MB/core — the memory roofline for this problem.
"""

import numpy as np

P = 128                    # SBUF partitions
DIM = 256                  # slot dim
R = 1024                   # hidden dim
E = 64                     # num experts
NCORES = 8
EPC = E // NCORES          # experts per core
DC = DIM // P              # layer-1 contraction chunks (2)
RC = R // P                # r chunks (8)
OC = DIM // P              # output dim chunks (2)
WCOLS = DC * R + RC * DIM  # weight columns per expert (4096)
BPE = RC + OC              # bias columns per expert (10)

USE_BF16 = False

_GRAPH_CACHE: dict = {}


def _build_graph(C: int, use_bf16: bool):
    import concourse.bacc as bacc
    import concourse.tile as tile
    from concourse import mybir

    f32 = mybir.dt.float32
    cdt = mybir.dt.bfloat16 if use_bf16 else f32

    # SBUF budget shrinks as the pad capacity C grows (pathological skew).
    if C <= 128:
        wg_bufs, h_bufs = 5, 2
    elif C <= 512:
        wg_bufs, h_bufs = 2, 2
    else:
        wg_bufs, h_bufs = 1, 1

    nc = bacc.Bacc(None, target_bir_lowering=False)
    xt_ext = nc.declare_dram_parameter("xt", [P, DC * EPC * C], cdt, isOutput=False)
    wg_ext = nc.declare_dram_parameter("wg", [EPC, P, WCOLS], cdt, isOutput=False)
    bias_ext = nc.declare_dram_parameter("bias", [P, EPC * BPE], f32, isOutput=False)
    out_ext = nc.declare_dram_parameter("out", [P, EPC * OC * C], f32, isOutput=True)

    mx = mybir.AluOpType.max
    aa = mybir.AluOpType.add
    ident = mybir.ActivationFunctionType.Identity
    relu = mybir.ActivationFunctionType.Relu

    with tile.TileContext(nc) as tc:
        with (
            tc.tile_pool(name="wpool", bufs=wg_bufs) as wpool,
            tc.tile_pool(name="xpool", bufs=1) as xpool,
            tc.tile_pool(name="hpool", bufs=h_bufs) as hpool,
            tc.tile_pool(name="opool", bufs=2) as opool,
            tc.tile_pool(name="ps1pool", bufs=5, space="PSUM") as ps1pool,
            tc.tile_pool(name="ps2pool", bufs=3, space="PSUM") as ps2pool,
        ):
            # Dummy activation up front so the 1.5us ACT_TABLE_LOAD the
            # compiler hoists before the first ACTIVATE runs during the
            # DMA fill instead of on the first expert's critical path.
            scratch = xpool.tile([P, 1], f32)
            nc.vector.memset(scratch[:], 0.0)
            scratch2 = xpool.tile([P, 1], f32)
            nc.scalar.activation(scratch2[:], scratch[:], relu, bias=scratch[:, 0:1])

            # One HWDGE queue (Sync) for all loads: queue order is the
            # critical path — xt and bias first (tiny; first matmul's rhs
            # and the relu bias), then per-expert w1|w2 as separate tiles
            # so layer 1 only waits on w1's bytes.
            xt = xpool.tile([P, DC * EPC * C], cdt)
            nc.sync.dma_start(xt[:], xt_ext[:])
            bias = xpool.tile([P, EPC * BPE], f32)
            nc.sync.dma_start(bias[:], bias_ext[:])
            w1s, w2s = [], []
            for e in range(EPC):
                w1g = wpool.tile([P, DC * R], cdt, tag="w1g")
                nc.sync.dma_start(w1g[:], wg_ext[e, :, : DC * R])
                w2g = wpool.tile([P, RC * DIM], cdt, tag="w2g")
                nc.sync.dma_start(w2g[:], wg_ext[e, :, DC * R :])
                w1s.append(w1g)
                w2s.append(w2g)

            for e in range(EPC):
                w1g, w2g = w1s[e], w2s[e]
                h = hpool.tile([P, RC * C], cdt)
                out_sb = opool.tile([P, OC * C], f32)
                for c0 in range(0, C, 512):
                    cw = min(512, C - c0)
                    # layer 1: H^T[r,:] = sum_d W1[d, r-block] . xT[d, :]
                    for rc_i in range(RC):
                        ps = ps1pool.tile([P, cw], f32)
                        for dc_i in range(DC):
                            nc.tensor.matmul(
                                ps[:],
                                w1g[:, dc_i * R + rc_i * P : dc_i * R + rc_i * P + P],
                                xt[:, (dc_i * EPC + e) * C + c0 : (dc_i * EPC + e) * C + c0 + cw],
                                start=(dc_i == 0),
                                stop=(dc_i == DC - 1),
                            )
                        # relu(x + b1), alternating Vector / Scalar engines
                        hs = h[:, rc_i * C + c0 : rc_i * C + c0 + cw]
                        bs = bias[:, e * BPE + rc_i : e * BPE + rc_i + 1]
                        if rc_i % 2 == 0:
                            nc.vector.tensor_scalar(hs, ps[:], bs, 0.0, aa, mx)
                        else:
                            nc.scalar.activation(hs, ps[:], relu, bias=bs)
                    # layer 2: Out^T[dim,:] = sum_r W2[r, dim-block] . H^T[r, :]
                    for oc_i in range(OC):
                        ps2 = ps2pool.tile([P, cw], f32)
                        for rc_i in range(RC):
                            nc.tensor.matmul(
                                ps2[:],
                                w2g[:, rc_i * DIM + oc_i * P : rc_i * DIM + oc_i * P + P],
                                h[:, rc_i * C + c0 : rc_i * C + c0 + cw],
                                start=(rc_i == 0),
                                stop=(rc_i == RC - 1),
                            )
                        nc.scalar.activation(
                            out_sb[:, oc_i * C + c0 : oc_i * C + c0 + cw],
                            ps2[:],
                            ident,
                            bias=bias[:, e * BPE + RC + oc_i : e * BPE + RC + oc_i + 1],
                        )
                nc.gpsimd.dma_start(out_ext[:, e * OC * C : (e + 1) * OC * C], out_sb[:])
    nc.compile()
    return nc


def _get_graph(C: int, use_bf16: bool):
    key = (C, use_bf16)
    if key not in _GRAPH_CACHE:
        _GRAPH_CACHE[key] = _build_graph(C, use_bf16)
    return _GRAPH_CACHE[key]


def _run(inputs: dict, trace: bool = False, trace_cores=None, use_bf16=None, **spmd_kwargs):
    from concourse.bass_utils import run_bass_kernel_spmd

    if use_bf16 is None:
        use_bf16 = USE_BF16
    if use_bf16:
        import ml_dtypes
        cdt_np = ml_dtypes.bfloat16
    else:
        cdt_np = np.float32

    slots = np.asarray(inputs["slots"], np.float32)
    w1 = np.asarray(inputs["w1"], np.float32)
    b1 = np.asarray(inputs["b1"], np.float32)
    w2 = np.asarray(inputs["w2"], np.float32)
    b2 = np.asarray(inputs["b2"], np.float32)
    indices = np.asarray(inputs["indices"]).astype(np.int64)

    B, K, D = slots.shape
    assert D == DIM and w1.shape == (E, DIM, R) and w2.shape == (E, R, DIM)
    X = slots.reshape(B * K, DIM)
    idx = indices.reshape(B * K)

    counts = np.bincount(idx, minlength=E)
    C = max(int(counts.max()), 16)
    C = ((C + 15) // 16) * 16  # stable capacities -> stable NEFF cache keys

    in_maps = []
    pos_lists = []
    for core in range(NCORES):
        xt = np.zeros((P, DC * EPC * C), cdt_np)
        wg = np.empty((EPC, P, WCOLS), cdt_np)
        bias = np.zeros((P, EPC * BPE), np.float32)
        core_pos = []
        for e in range(EPC):
            g = core * EPC + e
            pos = np.nonzero(idx == g)[0]
            core_pos.append(pos)
            n = len(pos)
            if n:
                xeT = X[pos].T.astype(cdt_np)  # [DIM, n]
                for dc_i in range(DC):
                    xt[:, (dc_i * EPC + e) * C : (dc_i * EPC + e) * C + n] = (
                        xeT[dc_i * P : (dc_i + 1) * P]
                    )
            wg[e, :, : DC * R] = (
                w1[g].reshape(DC, P, R).transpose(1, 0, 2).reshape(P, DC * R)
            )
            wg[e, :, DC * R :] = (
                w2[g].reshape(RC, P, DIM).transpose(1, 0, 2).reshape(P, RC * DIM)
            )
            bias[:, e * BPE : e * BPE + RC] = b1[g].reshape(RC, P).T
            bias[:, e * BPE + RC : (e + 1) * BPE] = b2[g].reshape(OC, P).T
        in_maps.append({"xt": xt, "wg": wg, "bias": bias})
        pos_lists.append(core_pos)

    nc = _get_graph(C, use_bf16)
    res = run_bass_kernel_spmd(
        nc, in_maps, core_ids=list(range(NCORES)), trace=trace,
        trace_cores=trace_cores, **spmd_kwargs,
    )

    out_flat = np.zeros((B * K, DIM), np.float32)
    for core in range(NCORES):
        o = res.results[core]["out"]  # [P, EPC*OC*C]
        for e in range(EPC):
            pos = pos_lists[core][e]
            n = len(pos)
            if n == 0:
                continue
            blk = np.empty((n, DIM), np.float32)
            for oc_i in range(OC):
                cols = o[:, (e * OC + oc_i) * C : (e * OC + oc_i) * C + n]
                blk[:, oc_i * P : (oc_i + 1) * P] = cols.T
            out_flat[pos] = blk
    return out_flat.reshape(B, K, DIM), res


def kernel(**inputs) -> np.ndarray:
    out, _ = _run(inputs)
    return out
